# revision 23
# baseline (speedup 1.0000x reference)
"""Trainium2 Bass kernel for nn_MoE_38242388803777 (top-1 routed MoE + shared FFN).

Strategy (expert-parallel, per the sharding hint):
 - Routing (cosine sim + argmax over 8 experts) is recomputed on host with the
   exact same jax-on-CPU ops as the reference, then tokens are dispatched: core
   e receives the tokens routed to expert e (padded to a fixed capacity) plus
   expert e's weights. The reference computes all 8 experts densely for every
   token; only the argmax expert contributes, so the kernel does 1/8 the FLOPs.
 - The shared (dense) FFN is data-parallel: core e also computes the shared
   expert for batch slice e.
 - All device matmuls run in fp16 (fp32 PSUM accumulation), weights pre-cast
   and pre-tiled on host into contiguous [128 x 128] lhsT tiles. Activations
   stay transposed (feature dim on partitions, tokens on the free dim) through
   the whole FFN so no on-device transposes are needed.
 - Host scatters the per-expert outputs back by routing index and adds the
   shared-slice outputs (pure unshard/combine; all math that touches the
   hidden dims runs on device).
"""

import numpy as np
from contextlib import ExitStack

import concourse.bass as bass
import concourse.mybir as mybir
import concourse.tile as tile
from concourse import bacc
from concourse.bass_utils import run_bass_kernel_spmd

B, DIM, INTER, E, LANG, SHARED_H = 2048, 1024, 2048, 8, 768, 4096
NC = 8
SB = B // NC            # shared-expert tokens per core

F16 = mybir.dt.float16
F32 = mybir.dt.float32
F8 = mybir.dt.float8e4
NP16 = np.float16
NP8 = mybir.dt.np(F8)
FP8A = True        # routed gate/up matmuls in fp8e4m3 + DoubleRow
FP8B = True        # routed down matmul in fp8e4m3 + DoubleRow
A1 = 16.0          # fp8 weight pre-scale (undone exactly in fp32 at eviction)

KD = DIM // 128         # 8  k-tiles over DIM
MI = INTER // 128       # 16 m-tiles over INTER
MS = SHARED_H // 128    # 32 m-tiles over SHARED_H
MD = DIM // 128         # 8  m-tiles over DIM
KI = INTER // 128       # 16 k-tiles over INTER
KS = SHARED_H // 128    # 32 k-tiles over SHARED_H


def _build(cap: int, fp8a: bool = FP8A, fp8b: bool = FP8B):
    """Build + compile the single-core SPMD program for routed capacity `cap`."""
    nc = bacc.Bacc("TRN2", target_bir_lowering=False, debug=False, num_devices=NC)

    if fp8a:
        xr_d = nc.dram_tensor("xr", [128, KD // 2, 2, cap], F8, kind="ExternalInput")
        wg_d = nc.dram_tensor("wg", [MI, 128, KD, 2, 128], F8, kind="ExternalInput")
    else:
        xr_d = nc.dram_tensor("xr", [128, KD, cap], F16, kind="ExternalInput")
        wg_d = nc.dram_tensor("wg", [MI, 128, 2 * KD, 128], F16, kind="ExternalInput")
    xs_d = nc.dram_tensor("xs", [128, KD, SB], F16, kind="ExternalInput")
    if fp8b:
        w2_d = nc.dram_tensor("w2", [MD, 128, KI // 2, 2, 128], F8, kind="ExternalInput")
        sc2_d = nc.dram_tensor("sc2", [128, 1], F32, kind="ExternalInput")
    else:
        w2_d = nc.dram_tensor("w2", [MD, 128, KI, 128], F16, kind="ExternalInput")
    ws1_d = nc.dram_tensor("ws1", [MS, 128, KD, 128], F16, kind="ExternalInput")
    ws2_d = nc.dram_tensor("ws2", [MD, 128, KS, 128], F16, kind="ExternalInput")
    b1_d = nc.dram_tensor("b1", [128, MI], F32, kind="ExternalInput")
    b3_d = nc.dram_tensor("b3", [128, MI], F32, kind="ExternalInput")
    b2_d = nc.dram_tensor("b2", [128, MD], F32, kind="ExternalInput")
    bs1_d = nc.dram_tensor("bs1", [128, MS], F32, kind="ExternalInput")
    bs2_d = nc.dram_tensor("bs2", [128, MD], F32, kind="ExternalInput")
    yr_d = nc.dram_tensor("yr", [MD, 128, cap], F32, kind="ExternalOutput")
    ys_d = nc.dram_tensor("ys", [MD, 128, SB], F32, kind="ExternalOutput")

    Silu = mybir.ActivationFunctionType.Silu
    Copy = mybir.ActivationFunctionType.Copy
    ADD = mybir.AluOpType.add
    MUL = mybir.AluOpType.mult

    with tile.TileContext(nc) as tc, ExitStack() as ctx:
        cpool = ctx.enter_context(tc.tile_pool(name="const", bufs=1))
        wpool = ctx.enter_context(tc.tile_pool(name="w", bufs=10))
        epool = ctx.enter_context(tc.tile_pool(name="evict", bufs=4))
        hpool = ctx.enter_context(tc.tile_pool(name="h", bufs=1))
        ppool = ctx.enter_context(tc.tile_pool(name="psum", bufs=7, space="PSUM"))

        # routed-phase constants first so the first weight DMAs aren't
        # queued behind inputs that later phases need
        # PE warmup: dummy matmuls with no input deps start immediately and
        # flip the HAM clock gate to 2.4GHz while the first DMAs land.
        warm_x = cpool.tile([128, 512], F16, tag="warmx")
        nc.gpsimd.memset(warm_x[:], 0.0)
        warm_ps = ppool.tile([128, 512], F32, tag="warmp", name="warm_ps", bufs=1)
        for i in range(12):
            nc.tensor.matmul(warm_ps[:], warm_x[:, :128], warm_x[:],
                             start=True, stop=True)

        # First-tile data split into small chunks, interleaved, so the PE's
        # first matmul only waits on ~100KB instead of ~1MB.
        if fp8a:
            xr_t = cpool.tile([128, KD // 2, 2, cap], F8, tag="xr")
            wg0_t = wpool.tile([128, KD, 2, 128], F8, tag="wg0", name="wg0", bufs=1)
            for i in range(4):
                nc.sync.dma_start(out=xr_t[:, i, :, :], in_=xr_d[:, i, :, :])
                nc.sync.dma_start(out=wg0_t[:, 2 * i : 2 * i + 2, :, :],
                                  in_=wg_d[0, :, 2 * i : 2 * i + 2, :, :])
        else:
            xr_t = cpool.tile([128, KD, cap], F16, tag="xr")
            wg0_t = wpool.tile([128, 2 * KD, 128], F16, tag="wg0", name="wg0", bufs=1)
            for i in range(4):
                nc.sync.dma_start(out=xr_t[:, 2 * i : 2 * i + 2, :],
                                  in_=xr_d[:, 2 * i : 2 * i + 2, :])
                nc.sync.dma_start(out=wg0_t[:, 4 * i : 4 * i + 4, :],
                                  in_=wg_d[0, :, 4 * i : 4 * i + 4, :])
        b1_t = cpool.tile([128, MI], F32, tag="b1")
        nc.sync.dma_start(out=b1_t[:], in_=b1_d[:])
        b3_t = cpool.tile([128, MI], F32, tag="b3")
        nc.sync.dma_start(out=b3_t[:], in_=b3_d[:])

        if fp8b:
            h_t = hpool.tile([128, KI // 2, 2, cap], F8, tag="h")  # silu(h1)*h3
        else:
            h_t = hpool.tile([128, KI, cap], F16, tag="h")    # silu(h1)*h3
        hs_t = hpool.tile([128, KS, SB], F16, tag="hs")   # silu(shared hidden)

        # ---- Phase A: routed gate/up matmuls (h = silu(x@W1+b1) * (x@W3+b3)) ----
        DR = mybir.MatmulPerfMode.DoubleRow
        for m in range(MI):
            if m == 0:
                wg_t = wg0_t
            elif fp8a:
                wg_t = wpool.tile([128, KD, 2, 128], F8, tag="w", name=f"wg{m}")
                nc.sync.dma_start(out=wg_t[:], in_=wg_d[m])
            else:
                wg_t = wpool.tile([128, 2 * KD, 128], F16, tag="w", name=f"wg{m}")
                nc.sync.dma_start(out=wg_t[:], in_=wg_d[m])
            ps1 = ppool.tile([128, cap], F32, tag="ps", name=f"ps1_{m}")
            ps3 = ppool.tile([128, cap], F32, tag="ps", name=f"ps3_{m}")
            if fp8a:
                KH = KD // 2
                for g in range(KH):
                    nc.tensor.matmul(ps1[:], wg_t[:, g, :, :], xr_t[:, g, :, :],
                                     start=(g == 0), stop=(g == KH - 1),
                                     perf_mode=DR)
                for g in range(KH):
                    nc.tensor.matmul(ps3[:], wg_t[:, KH + g, :, :], xr_t[:, g, :, :],
                                     start=(g == 0), stop=(g == KH - 1),
                                     perf_mode=DR)
            else:
                for k in range(KD):
                    nc.tensor.matmul(ps1[:], wg_t[:, k, :], xr_t[:, k, :],
                                     start=(k == 0), stop=(k == KD - 1))
                for k in range(KD):
                    nc.tensor.matmul(ps3[:], wg_t[:, KD + k, :], xr_t[:, k, :],
                                     start=(k == 0), stop=(k == KD - 1))
            h1s = epool.tile([128, cap], F16, tag="ev16", name=f"h1s{m}")
            nc.scalar.activation(h1s[:], ps1[:], Silu, bias=b1_t[:, m : m + 1],
                                 scale=(1.0 / A1) if fp8a else 1.0)
            if fp8b:
                # b3_d carries A1*b3, so this yields A1*h; the second op
                # rescales to h for the fp8 store (A1*h can exceed fp8 max).
                ha = epool.tile([128, cap], F16, tag="ev16b", name=f"ha{m}")
                nc.vector.scalar_tensor_tensor(
                    out=ha[:], in0=ps3[:], scalar=b3_t[:, m : m + 1],
                    in1=h1s[:], op0=ADD, op1=MUL)
                nc.vector.tensor_scalar_mul(
                    h_t[:, m // 2, m % 2, :], ha[:], 1.0 / A1)
            else:
                nc.vector.scalar_tensor_tensor(
                    out=h_t[:, m, :], in0=ps3[:], scalar=b3_t[:, m : m + 1],
                    in1=h1s[:], op0=ADD, op1=MUL)

        # ---- Phase B: routed down matmul (y = h@W2s + b2s; W2s pre-scaled) ----
        xs_t = cpool.tile([128, KD, SB], F16, tag="xs")
        nc.sync.dma_start(out=xs_t[:], in_=xs_d[:])
        b2_t = cpool.tile([128, MD], F32, tag="b2")
        nc.sync.dma_start(out=b2_t[:], in_=b2_d[:])
        if fp8b:
            sc2_t = cpool.tile([128, 1], F32, tag="sc2")
            nc.sync.dma_start(out=sc2_t[:], in_=sc2_d[:])
            w2_0t = wpool.tile([128, KI // 2, 2, 128], F8, tag="wb0", name="w2_0t", bufs=1)
        else:
            w2_0t = wpool.tile([128, KI, 128], F16, tag="wb0", name="w2_0t", bufs=1)
        nc.sync.dma_start(out=w2_0t[:], in_=w2_d[0])
        ws1_pre = []
        for i in range(4):
            t = wpool.tile([128, KD, 128], F16, tag="wc0", name=f"ws1_p{i}", bufs=4)
            nc.sync.dma_start(out=t[:], in_=ws1_d[i])
            ws1_pre.append(t)
        for m in range(MD):
            if m == 0:
                w2_t = w2_0t
            elif fp8b:
                w2_t = wpool.tile([128, KI // 2, 2, 128], F8, tag="w", name=f"w2_{m}")
                nc.sync.dma_start(out=w2_t[:], in_=w2_d[m])
            else:
                w2_t = wpool.tile([128, KI, 128], F16, tag="w", name=f"w2_{m}")
                nc.sync.dma_start(out=w2_t[:], in_=w2_d[m])
            ps = ppool.tile([128, cap], F32, tag="ps", name=f"psb_{m}")
            if fp8b:
                KH2 = KI // 2
                for g in range(KH2):
                    nc.tensor.matmul(ps[:], w2_t[:, g, :, :], h_t[:, g, :, :],
                                     start=(g == 0), stop=(g == KH2 - 1),
                                     perf_mode=DR)
            else:
                for k in range(KI):
                    nc.tensor.matmul(ps[:], w2_t[:, k, :], h_t[:, k, :],
                                     start=(k == 0), stop=(k == KI - 1))
            y_t = epool.tile([128, cap], F32, tag="ev32", name=f"y{m}")
            if fp8b:
                # y = ps * (sc/A2) + sc*b2   (sc2_t carries sc/A2; b2_d carries sc*b2)
                nc.vector.tensor_scalar(
                    out=y_t[:], in0=ps[:], scalar1=sc2_t[:, 0:1],
                    scalar2=b2_t[:, m : m + 1], op0=MUL, op1=ADD)
            else:
                nc.vector.tensor_scalar_add(y_t[:], ps[:], b2_t[:, m : m + 1])
            nc.scalar.dma_start(out=yr_d[m], in_=y_t[:])

        # ---- Phase C: shared up matmul (hs = silu(x@Ws1+bs1)) ----
        bs1_t = cpool.tile([128, MS], F32, tag="bs1")
        nc.sync.dma_start(out=bs1_t[:], in_=bs1_d[:])
        ws2_0t = wpool.tile([128, KS, 128], F16, tag="wd0", name="ws2_0t", bufs=1)
        nc.sync.dma_start(out=ws2_0t[:], in_=ws2_d[0])
        for m in range(MS):
            if m < 4:
                ws1_t = ws1_pre[m]
            else:
                ws1_t = wpool.tile([128, KD, 128], F16, tag="w", name=f"ws1_{m}")
                eng = nc.sync if m % 2 == 0 else nc.gpsimd
                eng.dma_start(out=ws1_t[:], in_=ws1_d[m])
            ps = ppool.tile([128, SB], F32, tag="ps", name=f"psc_{m}")
            for k in range(KD):
                nc.tensor.matmul(ps[:], ws1_t[:, k, :], xs_t[:, k, :],
                                 start=(k == 0), stop=(k == KD - 1))
            nc.scalar.activation(hs_t[:, m, :], ps[:], Silu, bias=bs1_t[:, m : m + 1])

        # ---- Phase D: shared down matmul (z = hs@Ws2 + bs2) ----
        bs2_t = cpool.tile([128, MD], F32, tag="bs2")
        nc.sync.dma_start(out=bs2_t[:], in_=bs2_d[:])
        for m in range(MD):
            if m == 0:
                ws2_t = ws2_0t
            else:
                ws2_t = wpool.tile([128, KS, 128], F16, tag="w", name=f"ws2_{m}")
                nc.sync.dma_start(out=ws2_t[:], in_=ws2_d[m])
            ps = ppool.tile([128, SB], F32, tag="ps", name=f"psd_{m}")
            for k in range(KS):
                nc.tensor.matmul(ps[:], ws2_t[:, k, :], hs_t[:, k, :],
                                 start=(k == 0), stop=(k == KS - 1))
            z_t = epool.tile([128, SB], F32, tag="ev32", name=f"z{m}")
            nc.vector.tensor_scalar_add(z_t[:], ps[:], bs2_t[:, m : m + 1])
            nc.scalar.dma_start(out=ys_d[m], in_=z_t[:])

    nc.compile()
    return nc


_CACHE: dict[tuple, object] = {}


def _get_nc(cap: int):
    key = (cap, FP8A, FP8B)
    if key not in _CACHE:
        _CACHE[key] = _build(cap, FP8A, FP8B)
    return _CACHE[key]


def _routing_idx(language_token: np.ndarray, route_emb: np.ndarray) -> np.ndarray:
    """Replicate the reference's routing bit-exactly: same jax ops on CPU.

    The smallest top1-top2 cosine margin is ~1.7e-6, so the argmax must be
    computed with the reference's own ops to guarantee identical expert
    assignment. Falls back to numpy (same math) if jax is unavailable.
    """
    try:
        import jax
        import jax.numpy as jnp

        with jax.default_device(jax.devices("cpu")[0]):
            lt = jnp.asarray(np.asarray(language_token, np.float32))
            re_ = jnp.asarray(np.asarray(route_emb, np.float32))

            def _l2norm(v, axis=-1, eps=1e-12):
                n = jnp.linalg.norm(v, axis=axis, keepdims=True)
                return v / jnp.maximum(n, eps)

            sims = _l2norm(lt) @ _l2norm(re_).T
            return np.asarray(jnp.argmax(sims, axis=-1))
    except Exception:
        lt = np.asarray(language_token, np.float32)
        re_ = np.asarray(route_emb, np.float32)
        lt = lt / np.maximum(np.linalg.norm(lt, axis=-1, keepdims=True), 1e-12)
        re_ = re_ / np.maximum(np.linalg.norm(re_, axis=-1, keepdims=True), 1e-12)
        return (lt @ re_.T).argmax(-1)


def _tile_w(w: np.ndarray, kt: int, mt: int) -> np.ndarray:
    """(K, M) fp16 -> (mt, 128, kt, 128) with w_t[m,p,k,c] = w[128k+p, 128m+c]."""
    return np.ascontiguousarray(
        w.reshape(kt, 128, mt, 128).transpose(2, 1, 0, 3))


def _tile_x(x: np.ndarray, cap: int) -> np.ndarray:
    """(n, DIM) fp32 -> (128, KD, cap) fp16 transposed+padded (partition-major)."""
    n = x.shape[0]
    xp = np.zeros((cap, DIM), NP16)
    xp[:n] = x.astype(NP16)
    return np.ascontiguousarray(xp.T.reshape(KD, 128, cap).transpose(1, 0, 2))


def _tile_x8(x: np.ndarray, cap: int) -> np.ndarray:
    """(n, DIM) fp32 -> (128, KD//2, 2, cap) fp8 paired for DoubleRow."""
    n = x.shape[0]
    xp = np.zeros((cap, DIM), np.float32)
    xp[:n] = x
    xt = np.ascontiguousarray(xp.T).astype(NP8)          # (DIM, cap)
    return np.ascontiguousarray(
        xt.reshape(KD // 2, 2, 128, cap).transpose(2, 0, 1, 3))


def _tile_w8(w: np.ndarray, kt: int, mt: int) -> np.ndarray:
    """(K, M) fp8 -> (mt, 128, kt//2, 2, 128) paired lhsT tiles for DoubleRow."""
    return np.ascontiguousarray(
        w.reshape(kt // 2, 2, 128, mt, 128).transpose(3, 2, 0, 1, 4))


def _tile_b(b: np.ndarray, mt: int) -> np.ndarray:
    """(M,) -> (128, mt) with out[p, m] = b[128m+p]."""
    return np.ascontiguousarray(b.astype(np.float32).reshape(mt, 128).T)


def run(inputs: dict, trace: bool = False, trace_cores=None):
    x = np.asarray(inputs["x"], np.float32)
    idx = _routing_idx(inputs["language_token"], inputs["route_emb"])
    ew = np.asarray(inputs["expert_weights"], np.float32)
    counts = np.bincount(idx, minlength=E)
    cap = max(16, int(-(-int(counts.max()) // 16) * 16))
    nc = _get_nc(cap)

    # shared-expert arrays (same for all cores)
    ws1_h = _tile_w(np.asarray(inputs["Ws1"]).astype(NP16), KD, MS)
    ws2_h = _tile_w(np.asarray(inputs["Ws2"]).astype(NP16), KS, MD)
    bs1_h = _tile_b(np.asarray(inputs["bs1"]), MS)
    bs2_h = _tile_b(np.asarray(inputs["bs2"]), MD)

    tok_lists = [np.nonzero(idx == e)[0] for e in range(E)]
    in_maps = []
    for e in range(E):
        toks = tok_lists[e]
        scale = float(ew[e])
        if FP8A:
            w1 = (np.asarray(inputs["W1"][e]) * A1).astype(NP8)
            w3 = (np.asarray(inputs["W3"][e]) * A1).astype(NP8)
            wg = np.concatenate(
                [_tile_w8(w1, KD, MI), _tile_w8(w3, KD, MI)], axis=2)
            xr = _tile_x8(x[toks], cap)
            if FP8B:
                w2 = _tile_w8((np.asarray(inputs["W2"][e]) * A1).astype(NP8), KI, MD)
            else:
                w2 = _tile_w(
                    (np.asarray(inputs["W2"][e]) * (scale / A1)).astype(NP16), KI, MD)
            b3 = _tile_b(np.asarray(inputs["b3"][e]) * A1, MI)
        else:
            w1 = np.asarray(inputs["W1"][e]).astype(NP16)
            w3 = np.asarray(inputs["W3"][e]).astype(NP16)
            wg = np.concatenate(
                [_tile_w(w1, KD, MI), _tile_w(w3, KD, MI)], axis=2)
            xr = _tile_x(x[toks], cap)
            w2 = _tile_w((np.asarray(inputs["W2"][e]) * scale).astype(NP16), KI, MD)
            b3 = _tile_b(np.asarray(inputs["b3"][e]), MI)
        im = {
            "xr": xr,
            "xs": _tile_x(x[e * SB : (e + 1) * SB], SB),
            "wg": np.ascontiguousarray(wg),
            "w2": w2,
            "ws1": ws1_h,
            "ws2": ws2_h,
            "b1": _tile_b(np.asarray(inputs["b1"][e]), MI),
            "b3": b3,
            "b2": _tile_b(np.asarray(inputs["b2"][e]) * scale, MD),
            "bs1": bs1_h,
            "bs2": bs2_h,
        }
        if FP8B:
            im["sc2"] = np.full((128, 1), scale / A1, np.float32)
        in_maps.append(im)

    res = run_bass_kernel_spmd(
        nc, in_maps, list(range(NC)), trace=trace,
        **({"trace_cores": trace_cores} if trace_cores is not None else {}),
    )

    out = np.empty((B, DIM), np.float32)
    for e in range(E):
        toks = tok_lists[e]
        y = res.results[e]["yr"].reshape(DIM, cap)
        out[toks] = y[:, : len(toks)].T
    for e in range(E):
        z = res.results[e]["ys"].reshape(DIM, SB)
        out[e * SB : (e + 1) * SB] += z.T
    return out, res


def kernel(**inputs) -> np.ndarray:
    out, _ = run(inputs, trace=False)
    return out
